# revision 63
# baseline (speedup 1.0000x reference)
"""Multi-head attention (B=2, S=2048, D=1024, H=16) on 8 TRN2 NeuronCores.

Sharding (Megatron-style): heads are tensor-parallel across the 8 cores
(2 heads each, batch replicated). Wq/Wk/Wv are column-parallel (each core
gets its heads' 128 output rows), Wo is row-parallel (each core gets the
matching 128 input columns); each core computes a full-shape partial of
the output projection and the host sums the 8 partials (the row-parallel
all-reduce, done at unshard time).

Per-core kernel (fp16 operands, fp32 PSUM accumulation). The scalar
engine's exp stream (~135us over 128 tiles) is the throughput floor, so
the program is arranged to keep it fed without gaps:
  - one flat tile program; emission interleaves projection chunks with
    attention key-blocks so the first exp issues as soon as K/Q of
    chunk 0 exist, and batch 1's projections overlap batch 0's attention
  - pool tags are arranged so no FIFO couples unrelated stages: the
    "mm" PSUM tag holds projection tiles only; out-projection tiles
    share the second PV-accumulator bank's tag
  - PV runs head 0 during the chunk's t-loop and head 1 as a burst
    afterwards (P^T is kept in a per-chunk SBUF slab), so each head
    needs only one PSUM accumulator bank and the accumulator release
    chain stays short (one DVE copy)
  - finish work (normalize/out-proj) of chunk c is emitted after the
    scores of chunk c+1 so the PE's static instruction stream never
    head-of-line blocks on it
  - input DMAs ride the sync queue; output DMAs ride the gpsimd queue
    and read PSUM directly (no staging copy)

  QT/KT = (x @ W.T).T computed directly in [head-dim, seq] layout
  V transposed to [seq, head-dim] via PE transpose, augmented with a ones
    column so the PV matmul also produces the softmax denominator
  S_T   = K_block.T @ Q per 128-key block, both heads co-issued on
          disjoint PE row groups (K=64 each) via tile_position
  P_T   = exp(0.125 * S_T) on the scalar engine
  O_aug = V_aug.T @ P_T accumulated over key blocks ([65, 512]; row 64 is
          the denominator)
  y     = O_aug[0:64] * broadcast(1/denominator)
  out  += y_block.T @ Wo_slice.T  (fp32 partial, summed on host)

PSUM budget (8 banks): "mm" x2 bufs = 2; scores [128,2,512]f32 x2 = 4;
PV accumulator h0 = 1; PV accumulator h1 / out-proj shared tag = 1.
"""

from contextlib import ExitStack

import numpy as np

import concourse.bass as bass
import concourse.mybir as mybir
import concourse.tile as tile
from concourse import bacc
from concourse.masks import make_identity

F32 = mybir.dt.float32
F16 = mybir.dt.float16

B = 2
S = 2048
D = 1024
H_LOCAL = 2          # heads per core
BS = B * S           # 4096
NE = D // 128        # contraction tiles for the projections
CHUNK = 512          # query-chunk width
NCH = S // CHUNK     # chunks per batch element
NTB = S // 128       # key blocks per batch element
SCALE = 0.125        # 1/sqrt(head_dim)
N_CORES = 8
DVE_EXP_T = (4, 9, 14)   # key-blocks whose exp runs on the DVE


def _proj_kq(nc, pools, w_sb, qT, kT, xT, state, b, c):
    """K/Q projection for one 512-token chunk — the part that unblocks
    the exp stream (K for the next key-blocks' scores, Q for the chunk's
    own attention)."""
    x_pool, mm_ps = pools["x"], pools["mm_ps"]
    g = b * NCH + c
    cols = bass.ds(g * CHUNK, CHUNK)

    xt = x_pool.tile([128, NE, CHUNK], F16, tag="xt", name="xt")
    for e in range(NE):
        nc.sync.dma_start(out=xt[:, e, :], in_=xT[e * 128:(e + 1) * 128, cols])
    state[("xt", b, c)] = xt

    ps = {}
    for name in ("wk", "wq"):
        p = mm_ps.tile([128, CHUNK], F32, tag="mm", name=f"ps_{name}")
        for e in range(NE):
            nc.tensor.matmul(p[:], w_sb[name][:, e, :], xt[:, e, :],
                             start=(e == 0), stop=(e == NE - 1))
        ps[name] = p
    nc.vector.tensor_copy(kT[:, cols], ps["wk"][:])
    nc.vector.tensor_copy(qT[:, cols], ps["wq"][:])


def _proj_v(nc, pools, w_sb, ident, v_aug, state, b, c):
    """V projection + transpose for one chunk — deadline-soft (PV only
    needs it by its chunk's accumulation), so it can be emitted at lower
    priority to fill PE slack."""
    vt_pool, mm_ps = pools["vt"], pools["mm_ps"]
    g = b * NCH + c
    xt = state.pop(("xt", b, c))

    p = mm_ps.tile([128, CHUNK], F32, tag="mm", name="ps_wv")
    for e in range(NE):
        nc.tensor.matmul(p[:], w_sb["wv"][:, e, :], xt[:, e, :],
                         start=(e == 0), stop=(e == NE - 1))
    vt = vt_pool.tile([128, CHUNK], F16, tag="vt", name="vt")
    nc.vector.tensor_copy(vt[:], p[:])

    # Transpose V [head-dim, tok] -> [tok, head-dim] per 128-token block;
    # both heads co-issued on disjoint 64-row PE tiles. Each head gets its
    # own PSUM tile (separate banks — the co-issued pair must not write the
    # same bank), then one strided copy per head into v_aug.
    tr = [mm_ps.tile([128, 4, 64], F16, tag="mm", name=f"tr{h}")
          for h in range(H_LOCAL)]
    for j in range(CHUNK // 128):
        for h in range(H_LOCAL):
            nc.tensor.transpose(tr[h][:, j, :], vt[64 * h:64 * h + 64,
                                                   bass.ds(j * 128, 128)],
                                ident[64 * h:64 * h + 64, 0:64])
    for h in range(H_LOCAL):
        nc.vector.tensor_copy(
            v_aug[:, b * H_LOCAL + h, bass.ds(c * 4, 4), 0:64],
            tr[h][:, :, :])


def _proj_chunk(nc, pools, w_sb, ident, qT, kT, v_aug, xT, state, b, c):
    _proj_kq(nc, pools, w_sb, qT, kT, xT, state, b, c)
    _proj_v(nc, pools, w_sb, ident, v_aug, state, b, c)


def _attn_part(nc, pools, qT, kT, v_aug, state, b, c, t_lo, t_hi):
    """Key-blocks [t_lo, t_hi) of query chunk (b, c): scores -> exp into
    the chunk's P^T slab. Emission granularity follows K-chunk
    availability so the sc-tag FIFO never head-of-line blocks on a
    not-yet-projected K chunk."""
    pt_pool, sc_ps = pools["pt"], pools["sc_ps"]
    scols = bass.ds(b * S + c * CHUNK, CHUNK)
    if t_lo == 0:
        state[(b, c)] = {
            "slab": pt_pool.tile([128, NTB, H_LOCAL, CHUNK], F16,
                                 tag="pt", name="pt"),
        }
    st = state[(b, c)]
    for t in range(t_lo, t_hi):
        tcols = bass.ds(b * S + t * 128, 128)
        sc = sc_ps.tile([128, H_LOCAL, CHUNK], F32, tag="sc", name="sc")
        for h in range(H_LOCAL):
            hp = slice(64 * h, 64 * h + 64)
            nc.tensor.matmul(sc[:, h, :], kT[hp, tcols], qT[hp, scols],
                             start=True, stop=True,
                             tile_position=(64 * h, 0))
        nc.scalar.activation(st["slab"][:, t, :, :], sc[:],
                             mybir.ActivationFunctionType.Exp,
                             scale=SCALE)


def _attn_pv0(nc, pools, v_aug, state, b, c):
    """Head-0 PV accumulation for chunk (b, c): emitted right after the
    chunk's exps, runs incrementally as the slab fills."""
    o_ps = pools["o_ps"]
    st = state[(b, c)]
    st["o0"] = o_ps.tile([65, CHUNK], F32, tag="o_h0", name="o_h0")
    for t in range(NTB):
        nc.tensor.matmul(st["o0"][:], v_aug[:, b * H_LOCAL, t, :],
                         st["slab"][:, t, 0, :],
                         start=(t == 0), stop=(t == NTB - 1))


def _normalize(nc, pools, osb, y_cT, h, scols):
    nrm_pool = pools["nrm"]
    rs = nrm_pool.tile([1, CHUNK], F32, tag="rs", name="rs")
    nc.vector.tensor_copy(rs[:], osb[64:65, :])
    rr = nrm_pool.tile([1, CHUNK], F32, tag="rr", name="rr")
    nc.vector.reciprocal_approx_fast(out=rr[:], in_=rs[:])
    bcr = nrm_pool.tile([64, CHUNK], F32, tag="bcr", name="bcr")
    nc.gpsimd.partition_broadcast(bcr[:], rr[:])
    nc.vector.tensor_mul(y_cT[64 * h:64 * h + 64, scols],
                         osb[0:64, :], bcr[:])


def _attn_pv1(nc, pools, v_aug, state, b, c):
    """Head-1 PV accumulation emitted early (only used for the final
    chunk): runs incrementally as the slab fills instead of as a burst
    after the last exp, shortening the kernel tail."""
    o_ps = pools["o_ps"]
    st = state[(b, c)]
    st["o1"] = o_ps.tile([65, CHUNK], F32, tag="po_o1", name="o_h1")
    for t in range(NTB):
        nc.tensor.matmul(st["o1"][:], v_aug[:, b * H_LOCAL + 1, t, :],
                         st["slab"][:, t, 1, :],
                         start=(t == 0), stop=(t == NTB - 1))


def _attn_finish(nc, pools, wo_sb, v_aug, y_cT, out, state, b, c,
                 last=False, two_wide=False, do_oproj=True):
    """Head-1 PV burst from the P^T slab, softmax normalization for
    both heads, then the output projection for chunk (b, c)'s four
    blocks. The h1 accumulator and the out-projection tiles share the
    "po_o1" PSUM tag (lifetimes sequential: o1 -> po x8 -> next o1).
    The last chunk's out-projection runs through the then-idle score
    banks instead so the kernel tail isn't a serial matmul->cast
    ladder, and its h1 accumulator uses the freed h0 bank."""
    osb_pool, o_ps, sc_ps = pools["osb"], pools["o_ps"], pools["sc_ps"]
    scols = bass.ds(b * S + c * CHUNK, CHUNK)
    st = state.pop((b, c))

    # Free the h0 accumulator bank with one copy; normalize from SBUF.
    osb0 = osb_pool.tile([65, CHUNK], F32, tag="osb", name="osb0")
    nc.vector.tensor_copy(osb0[:], st["o0"][:])
    _normalize(nc, pools, osb0, y_cT, 0, scols)

    if "o1" in st:
        o1 = st["o1"]
    else:
        o1 = o_ps.tile([65, CHUNK], F32, tag="o_h0" if last else "po_o1",
                       name="o_h1")
        for t in range(NTB):
            nc.tensor.matmul(o1[:], v_aug[:, b * H_LOCAL + 1, t, :],
                             st["slab"][:, t, 1, :],
                             start=(t == 0), stop=(t == NTB - 1))
    osb1 = osb_pool.tile([65, CHUNK], F32, tag="osb", name="osb1")
    nc.vector.tensor_copy(osb1[:], o1[:])
    _normalize(nc, pools, osb1, y_cT, 1, scols)

    if do_oproj:
        _attn_oproj(nc, pools, wo_sb, y_cT, out, b, c,
                    last=last, two_wide=two_wide)


def _attn_oproj(nc, pools, wo_sb, y_cT, out, b, c, last=False,
                two_wide=False):
    """Output projection + DMA for chunk (b, c)'s four blocks. Deadline
    free — can be emitted long after the chunk to fill PE slack."""
    o_ps, sc_ps = pools["o_ps"], pools["sc_ps"]
    out_pool = pools["out"]
    for j in range(CHUNK // 128):
        rows = bass.ds(b * S + (c * 4 + j) * 128, 128)
        ot = out_pool.tile([128, D], F16, tag="ot", name="ot")
        for f in range(D // CHUNK):
            fcols = bass.ds(f * CHUNK, CHUNK)
            if last:
                po = sc_ps.tile([128, CHUNK], F32, tag="sc", name="po")
            elif two_wide and f == 0:
                # After the last projection the "mm" banks are idle;
                # batch 1's finishes ping-pong through them so the
                # matmul->cast chain is two-wide.
                po = pools["mm_ps"].tile([128, CHUNK], F32, tag="mm",
                                         name="po")
            else:
                po = o_ps.tile([128, CHUNK], F32, tag="po_o1", name="po")
            nc.tensor.matmul(po[:], y_cT[:, rows], wo_sb[:, fcols],
                             start=True, stop=True)
            if last:
                # The exp stream is over — the scalar engine is idle, so
                # it carries the tail's PSUM->SBUF casts while the DVE
                # runs the normalize chains.
                nc.scalar.copy(ot[:, fcols], po[:])
            else:
                nc.vector.tensor_copy(ot[:, fcols], po[:])
        if last:
            # The sync queue is idle by now and its drain is cheap —
            # the gpsimd queue's final drain is not.
            nc.sync.dma_start(out=out[rows, :], in_=ot[:])
        else:
            nc.gpsimd.dma_start(out=out[rows, :], in_=ot[:])


def _mha_kernel(tc, out, xT, wqT, wkT, wvT, woT):
    nc = tc.nc
    with ExitStack() as ctx:
        singles = ctx.enter_context(tc.tile_pool(name="singles", bufs=1))

        # Weights ride the scalar-engine DMA queue (4 triggers, issued
        # before any exp work) so they flow in parallel with the first x
        # chunk's DMAs on the sync queue.
        w_sb = {}
        for name, ap in (("wq", wqT), ("wk", wkT), ("wv", wvT)):
            t = singles.tile([128, NE, 128], F16, tag=f"w_{name}",
                             name=f"w_{name}")
            nc.scalar.dma_start(out=t[:],
                                in_=ap.rearrange("(e p) o -> p e o", p=128))
            w_sb[name] = t
        wo_sb = singles.tile([128, D], F16, tag="wo")
        nc.scalar.dma_start(out=wo_sb[:], in_=woT[:])

        ident = singles.tile([128, 64], F16, tag="ident")
        make_identity(nc, ident[0:64, 0:64])
        make_identity(nc, ident[64:128, 0:64])

        qT = singles.tile([128, BS], F16, tag="qT")
        kT = singles.tile([128, BS], F16, tag="kT")
        v_aug = singles.tile([128, B * H_LOCAL, NTB, 65], F16, tag="v_aug")
        ones = singles.tile([128, 1], F16, tag="ones")
        nc.vector.memset(ones[:], 1.0)
        nc.vector.tensor_copy(
            v_aug[:, :, :, 64:65],
            ones[:].to_broadcast((128, B * H_LOCAL, NTB, 1)))
        y_cT = singles.tile([128, BS], F16, tag="y_cT")

        pools = {
            "x": ctx.enter_context(tc.tile_pool(name="x_pool", bufs=3)),
            "vt": ctx.enter_context(tc.tile_pool(name="vt_pool", bufs=2)),
            "pt": ctx.enter_context(tc.tile_pool(name="pt_pool", bufs=3)),
            "osb": ctx.enter_context(tc.tile_pool(name="osb_pool", bufs=4)),
            "nrm": ctx.enter_context(tc.tile_pool(name="nrm_pool", bufs=2)),
            "out": ctx.enter_context(tc.tile_pool(name="out_pool", bufs=3)),
            "mm_ps": ctx.enter_context(
                tc.tile_pool(name="mm_ps", bufs=2, space="PSUM")),
            "sc_ps": ctx.enter_context(
                tc.tile_pool(name="sc_ps", bufs=2, space="PSUM")),
            "o_ps": ctx.enter_context(
                tc.tile_pool(name="o_ps", bufs=1, space="PSUM")),
        }

        state = {}

        def proj(b, c):
            _proj_chunk(nc, pools, w_sb, ident, qT, kT, v_aug, xT, state,
                        b, c)

        def kq(b, c):
            _proj_kq(nc, pools, w_sb, qT, kT, xT, state, b, c)

        def vproj(b, c):
            _proj_v(nc, pools, w_sb, ident, v_aug, state, b, c)

        def part(b, c, lo, hi):
            _attn_part(nc, pools, qT, kT, v_aug, state, b, c, lo, hi)

        def pv0(b, c):
            _attn_pv0(nc, pools, v_aug, state, b, c)

        def fin(b, c, last=False, two_wide=False, do_oproj=True):
            _attn_finish(nc, pools, wo_sb, v_aug, y_cT, out, state, b, c,
                         last=last, two_wide=two_wide, do_oproj=do_oproj)

        def oproj(b, c):
            _attn_oproj(nc, pools, wo_sb, y_cT, out, b, c, two_wide=True)

        # Warm the PE p-state ramp while the first DMAs are in flight:
        # identity transposes depend only on `ident` (no DMA), and a
        # dummy reduce releases the PSUM slot before projections need it.
        warm = pools["mm_ps"].tile([128, 16, 64], F16, tag="mm",
                                   name="warm")
        for w in range(16):
            nc.tensor.transpose(warm[0:64, w, :], ident[0:64, 0:64],
                                ident[0:64, 0:64])
        wsink = singles.tile([64, 16, 64], F16, tag="wsink")
        nc.vector.tensor_copy(wsink[:], warm[0:64, :, :])

        # Batch 0 pipeline fill: each projection chunk unlocks 4 key
        # blocks of chunk 0's attention plus borrowed exp-only blocks of
        # chunk 1, sized to keep the scalar engine saturated while the
        # next projection runs. Head-0 PV trails as soon as each chunk's
        # exps are all emitted; finish bursts trail one chunk; batch 1's
        # projections are spread through batch 0's attention.
        proj(0, 0)
        part(0, 0, 0, 4)
        proj(0, 1)
        part(0, 0, 4, 8)
        part(0, 1, 0, 5)
        proj(0, 2)
        part(0, 0, 8, 12)
        part(0, 1, 5, 9)
        proj(0, 3)
        part(0, 0, 12, 16)
        pv0(0, 0)
        part(0, 1, 9, 16)
        pv0(0, 1)
        # Batch 0's finishes skip their out-projections here: those are
        # deadline-free and deferred into batch 1's attention windows,
        # decongesting the PE around the batch transition.
        part(0, 2, 0, 16)
        pv0(0, 2)
        fin(0, 0, do_oproj=False)
        kq(1, 0)
        part(0, 3, 0, 16)
        pv0(0, 3)
        vproj(1, 0)
        fin(0, 1, do_oproj=False)
        kq(1, 1)
        vproj(1, 1)
        fin(0, 2, do_oproj=False)
        kq(1, 2)
        kq(1, 3)
        vproj(1, 2)
        vproj(1, 3)
        fin(0, 3, do_oproj=False)
        part(1, 0, 0, 16)
        pv0(1, 0)
        oproj(0, 0)
        part(1, 1, 0, 16)
        pv0(1, 1)
        oproj(0, 1)
        fin(1, 0, two_wide=True)
        oproj(0, 2)
        part(1, 2, 0, 16)
        pv0(1, 2)
        oproj(0, 3)
        fin(1, 1, two_wide=True)
        part(1, 3, 0, 16)
        pv0(1, 3)
        fin(1, 2, two_wide=True)
        fin(1, 3, last=True)


def build_nc(n_cores=N_CORES):
    nc = bacc.Bacc("TRN2", target_bir_lowering=False, debug=False,
                   num_devices=n_cores)
    xT = nc.dram_tensor("xT", [D, BS], F16, kind="ExternalInput").ap()
    wqT = nc.dram_tensor("wqT", [D, 128], F16, kind="ExternalInput").ap()
    wkT = nc.dram_tensor("wkT", [D, 128], F16, kind="ExternalInput").ap()
    wvT = nc.dram_tensor("wvT", [D, 128], F16, kind="ExternalInput").ap()
    woT = nc.dram_tensor("woT", [128, D], F16, kind="ExternalInput").ap()
    out = nc.dram_tensor("out", [BS, D], F16, kind="ExternalOutput").ap()
    with tile.TileContext(nc) as tc:
        _mha_kernel(tc, out, xT, wqT, wkT, wvT, woT)
    nc.compile()
    return nc


def make_in_maps(inputs, Wq, Wk, Wv, Wo, n_cores=N_CORES):
    x = np.asarray(inputs, dtype=np.float32).reshape(BS, D)
    xT = np.ascontiguousarray(x.T).astype(np.float16)
    Wq, Wk, Wv, Wo = (np.asarray(w, dtype=np.float32)
                      for w in (Wq, Wk, Wv, Wo))
    maps = []
    for c in range(n_cores):
        sl = slice(c * 128, (c + 1) * 128)
        maps.append({
            "xT": xT,
            "wqT": np.ascontiguousarray(Wq[sl, :].T).astype(np.float16),
            "wkT": np.ascontiguousarray(Wk[sl, :].T).astype(np.float16),
            "wvT": np.ascontiguousarray(Wv[sl, :].T).astype(np.float16),
            "woT": np.ascontiguousarray(Wo[:, sl].T).astype(np.float16),
        })
    return maps


_NC_CACHE = None


def run(inputs, Wq, Wk, Wv, Wo, trace=False):
    """Shard, run on the 8 NeuronCores, and unshard. Returns
    (output [B,S,D] float32, BassKernelResults)."""
    global _NC_CACHE
    from concourse.bass_utils import run_bass_kernel_spmd
    if _NC_CACHE is None:
        _NC_CACHE = build_nc()
    maps = make_in_maps(inputs, Wq, Wk, Wv, Wo)
    res = run_bass_kernel_spmd(_NC_CACHE, maps, list(range(N_CORES)),
                               trace=trace)
    acc = np.zeros((BS, D), dtype=np.float32)
    for rmap in res.results:
        acc += rmap["out"].astype(np.float32)
    return acc.reshape(B, S, D), res


def kernel(inputs, Wq, Wk, Wv, Wo):
    out, _ = run(inputs, Wq, Wk, Wv, Wo, trace=False)
    return out


# revision 64
# speedup vs baseline: 1.1937x; 1.1937x over previous
"""Multi-head attention (B=2, S=2048, D=1024, H=16) on 8 TRN2 NeuronCores.

Sharding (Megatron-style): heads are tensor-parallel across the 8 cores
(2 heads each, batch replicated). Wq/Wk/Wv are column-parallel (each core
gets its heads' 128 output rows), Wo is row-parallel (each core gets the
matching 128 input columns); each core computes a full-shape partial of
the output projection and the host sums the 8 partials (the row-parallel
all-reduce, done at unshard time).

Per-core kernel (fp16 operands, fp32 PSUM accumulation). The scalar
engine's exp stream (~135us over 128 tiles) is the throughput floor, so
the program is arranged to keep it fed without gaps:
  - one flat tile program; emission interleaves projection chunks with
    attention key-blocks so the first exp issues as soon as K/Q of
    chunk 0 exist, and batch 1's projections overlap batch 0's attention
  - pool tags are arranged so no FIFO couples unrelated stages: the
    "mm" PSUM tag holds projection tiles only; out-projection tiles
    share the second PV-accumulator bank's tag
  - PV runs head 0 during the chunk's t-loop and head 1 as a burst
    afterwards (P^T is kept in a per-chunk SBUF slab), so each head
    needs only one PSUM accumulator bank and the accumulator release
    chain stays short (one DVE copy)
  - finish work (normalize/out-proj) of chunk c is emitted after the
    scores of chunk c+1 so the PE's static instruction stream never
    head-of-line blocks on it
  - input DMAs ride the sync queue; output DMAs ride the gpsimd queue
    and read PSUM directly (no staging copy)

  QT/KT = (x @ W.T).T computed directly in [head-dim, seq] layout
  V transposed to [seq, head-dim] via PE transpose, augmented with a ones
    column so the PV matmul also produces the softmax denominator
  S_T   = K_block.T @ Q per 128-key block, both heads co-issued on
          disjoint PE row groups (K=64 each) via tile_position
  P_T   = exp(0.125 * S_T) on the scalar engine
  O_aug = V_aug.T @ P_T accumulated over key blocks ([65, 512]; row 64 is
          the denominator)
  y     = O_aug[0:64] * broadcast(1/denominator)
  out  += y_block.T @ Wo_slice.T  (fp32 partial, summed on host)

PSUM budget (8 banks): "mm" x2 bufs = 2; scores [128,2,512]f32 x2 = 4;
PV accumulator h0 = 1; PV accumulator h1 / out-proj shared tag = 1.
"""

from contextlib import ExitStack

import numpy as np

import concourse.bass as bass
import concourse.mybir as mybir
import concourse.tile as tile
from concourse import bacc
from concourse.masks import make_identity

F32 = mybir.dt.float32
F16 = mybir.dt.float16

B = 2
S = 2048
D = 1024
H_LOCAL = 2          # heads per core
BS = B * S           # 4096
NE = D // 128        # contraction tiles for the projections
CHUNK = 512          # query-chunk width
NCH = S // CHUNK     # chunks per batch element
NTB = S // 128       # key blocks per batch element
SCALE = 0.125        # 1/sqrt(head_dim)
N_CORES = 8
DVE_EXP_T = (4, 9, 14)   # key-blocks whose exp runs on the DVE


def _proj_kq(nc, pools, w_sb, qT, kT, xT, state, b, c):
    """K/Q projection for one 512-token chunk — the part that unblocks
    the exp stream (K for the next key-blocks' scores, Q for the chunk's
    own attention)."""
    x_pool, mm_ps = pools["x"], pools["mm_ps"]
    g = b * NCH + c
    cols = bass.ds(g * CHUNK, CHUNK)

    xt = x_pool.tile([128, NE, CHUNK], F16, tag="xt", name="xt")
    for e in range(NE):
        nc.sync.dma_start(out=xt[:, e, :], in_=xT[e * 128:(e + 1) * 128, cols])
    state[("xt", b, c)] = xt

    ps = {}
    for name in ("wk", "wq"):
        p = mm_ps.tile([128, CHUNK], F32, tag="mm", name=f"ps_{name}")
        for e in range(NE):
            nc.tensor.matmul(p[:], w_sb[name][:, e, :], xt[:, e, :],
                             start=(e == 0), stop=(e == NE - 1))
        ps[name] = p
    nc.vector.tensor_copy(kT[:, cols], ps["wk"][:])
    nc.vector.tensor_copy(qT[:, cols], ps["wq"][:])


def _proj_v(nc, pools, w_sb, ident, v_aug, state, b, c):
    """V projection + transpose for one chunk — deadline-soft (PV only
    needs it by its chunk's accumulation), so it can be emitted at lower
    priority to fill PE slack."""
    vt_pool, mm_ps = pools["vt"], pools["mm_ps"]
    g = b * NCH + c
    xt = state.pop(("xt", b, c))

    p = mm_ps.tile([128, CHUNK], F32, tag="mm", name="ps_wv")
    for e in range(NE):
        nc.tensor.matmul(p[:], w_sb["wv"][:, e, :], xt[:, e, :],
                         start=(e == 0), stop=(e == NE - 1))
    vt = vt_pool.tile([128, CHUNK], F16, tag="vt", name="vt")
    nc.vector.tensor_copy(vt[:], p[:])

    # Transpose V [head-dim, tok] -> [tok, head-dim] per 128-token block;
    # both heads co-issued on disjoint 64-row PE tiles. Each head gets its
    # own PSUM tile (separate banks — the co-issued pair must not write the
    # same bank), then one strided copy per head into v_aug.
    tr = [mm_ps.tile([128, 4, 64], F16, tag="mm", name=f"tr{h}")
          for h in range(H_LOCAL)]
    for j in range(CHUNK // 128):
        for h in range(H_LOCAL):
            nc.tensor.transpose(tr[h][:, j, :], vt[64 * h:64 * h + 64,
                                                   bass.ds(j * 128, 128)],
                                ident[64 * h:64 * h + 64, 0:64])
    for h in range(H_LOCAL):
        nc.vector.tensor_copy(
            v_aug[:, b * H_LOCAL + h, bass.ds(c * 4, 4), 0:64],
            tr[h][:, :, :])


def _proj_chunk(nc, pools, w_sb, ident, qT, kT, v_aug, xT, state, b, c):
    _proj_kq(nc, pools, w_sb, qT, kT, xT, state, b, c)
    _proj_v(nc, pools, w_sb, ident, v_aug, state, b, c)


def _attn_part(nc, pools, qT, kT, v_aug, state, b, c, t_lo, t_hi):
    """Key-blocks [t_lo, t_hi) of query chunk (b, c): scores -> exp into
    the chunk's P^T slab. Emission granularity follows K-chunk
    availability so the sc-tag FIFO never head-of-line blocks on a
    not-yet-projected K chunk."""
    pt_pool, sc_ps = pools["pt"], pools["sc_ps"]
    scols = bass.ds(b * S + c * CHUNK, CHUNK)
    if t_lo == 0:
        state[(b, c)] = {
            "slab": pt_pool.tile([128, NTB, H_LOCAL, CHUNK], F16,
                                 tag="pt", name="pt"),
        }
    st = state[(b, c)]
    for t in range(t_lo, t_hi):
        tcols = bass.ds(b * S + t * 128, 128)
        sc = sc_ps.tile([128, H_LOCAL, CHUNK], F32, tag="sc", name="sc")
        for h in range(H_LOCAL):
            hp = slice(64 * h, 64 * h + 64)
            nc.tensor.matmul(sc[:, h, :], kT[hp, tcols], qT[hp, scols],
                             start=True, stop=True,
                             tile_position=(64 * h, 0))
        nc.scalar.activation(st["slab"][:, t, :, :], sc[:],
                             mybir.ActivationFunctionType.Exp,
                             scale=SCALE)


def _attn_pv0(nc, pools, v_aug, state, b, c):
    """Head-0 PV accumulation for chunk (b, c): emitted right after the
    chunk's exps, runs incrementally as the slab fills."""
    o_ps = pools["o_ps"]
    st = state[(b, c)]
    st["o0"] = o_ps.tile([65, CHUNK], F32, tag="o_h0", name="o_h0")
    for t in range(NTB):
        nc.tensor.matmul(st["o0"][:], v_aug[:, b * H_LOCAL, t, :],
                         st["slab"][:, t, 0, :],
                         start=(t == 0), stop=(t == NTB - 1))


def _normalize(nc, pools, osb, y_cT, h, scols):
    nrm_pool = pools["nrm"]
    rs = nrm_pool.tile([1, CHUNK], F32, tag="rs", name="rs")
    nc.vector.tensor_copy(rs[:], osb[64:65, :])
    rr = nrm_pool.tile([1, CHUNK], F32, tag="rr", name="rr")
    nc.vector.reciprocal_approx_fast(out=rr[:], in_=rs[:])
    bcr = nrm_pool.tile([64, CHUNK], F32, tag="bcr", name="bcr")
    nc.gpsimd.partition_broadcast(bcr[:], rr[:])
    nc.vector.tensor_mul(y_cT[64 * h:64 * h + 64, scols],
                         osb[0:64, :], bcr[:])


def _attn_pv1(nc, pools, v_aug, state, b, c):
    """Head-1 PV accumulation emitted early (only used for the final
    chunk): runs incrementally as the slab fills instead of as a burst
    after the last exp, shortening the kernel tail."""
    o_ps = pools["o_ps"]
    st = state[(b, c)]
    st["o1"] = o_ps.tile([65, CHUNK], F32, tag="po_o1", name="o_h1")
    for t in range(NTB):
        nc.tensor.matmul(st["o1"][:], v_aug[:, b * H_LOCAL + 1, t, :],
                         st["slab"][:, t, 1, :],
                         start=(t == 0), stop=(t == NTB - 1))


def _attn_finish(nc, pools, wo_sb, v_aug, y_cT, out, state, b, c,
                 last=False, two_wide=False, do_oproj=True):
    """Head-1 PV burst from the P^T slab, softmax normalization for
    both heads, then the output projection for chunk (b, c)'s four
    blocks. The h1 accumulator and the out-projection tiles share the
    "po_o1" PSUM tag (lifetimes sequential: o1 -> po x8 -> next o1).
    The last chunk's out-projection runs through the then-idle score
    banks instead so the kernel tail isn't a serial matmul->cast
    ladder, and its h1 accumulator uses the freed h0 bank."""
    osb_pool, o_ps, sc_ps = pools["osb"], pools["o_ps"], pools["sc_ps"]
    scols = bass.ds(b * S + c * CHUNK, CHUNK)
    st = state.pop((b, c))

    # Free the h0 accumulator bank with one copy; normalize from SBUF.
    osb0 = osb_pool.tile([65, CHUNK], F32, tag="osb", name="osb0")
    nc.vector.tensor_copy(osb0[:], st["o0"][:])
    _normalize(nc, pools, osb0, y_cT, 0, scols)

    if "o1" in st:
        o1 = st["o1"]
    else:
        o1 = o_ps.tile([65, CHUNK], F32, tag="o_h0" if last else "po_o1",
                       name="o_h1")
        for t in range(NTB):
            nc.tensor.matmul(o1[:], v_aug[:, b * H_LOCAL + 1, t, :],
                             st["slab"][:, t, 1, :],
                             start=(t == 0), stop=(t == NTB - 1))
    osb1 = osb_pool.tile([65, CHUNK], F32, tag="osb", name="osb1")
    nc.vector.tensor_copy(osb1[:], o1[:])
    _normalize(nc, pools, osb1, y_cT, 1, scols)

    if do_oproj:
        _attn_oproj(nc, pools, wo_sb, y_cT, out, b, c,
                    last=last, two_wide=two_wide)


def _attn_oproj(nc, pools, wo_sb, y_cT, out, b, c, last=False,
                two_wide=False):
    """Output projection + DMA for chunk (b, c)'s four blocks. Deadline
    free — can be emitted long after the chunk to fill PE slack."""
    o_ps, sc_ps = pools["o_ps"], pools["sc_ps"]
    out_pool = pools["out"]
    for j in range(CHUNK // 128):
        rows = bass.ds(b * S + (c * 4 + j) * 128, 128)
        ot = out_pool.tile([128, D], F16, tag="ot", name="ot")
        for f in range(D // CHUNK):
            fcols = bass.ds(f * CHUNK, CHUNK)
            if last:
                po = sc_ps.tile([128, CHUNK], F32, tag="sc", name="po")
            elif two_wide and f == 0:
                # After the last projection the "mm" banks are idle;
                # batch 1's finishes ping-pong through them so the
                # matmul->cast chain is two-wide.
                po = pools["mm_ps"].tile([128, CHUNK], F32, tag="mm",
                                         name="po")
            else:
                po = o_ps.tile([128, CHUNK], F32, tag="po_o1", name="po")
            nc.tensor.matmul(po[:], y_cT[:, rows], wo_sb[:, fcols],
                             start=True, stop=True)
            if last:
                # The exp stream is over — the scalar engine is idle, so
                # it carries the tail's PSUM->SBUF casts while the DVE
                # runs the normalize chains.
                nc.scalar.copy(ot[:, fcols], po[:])
            else:
                nc.vector.tensor_copy(ot[:, fcols], po[:])
        if last:
            # The sync queue is idle by now and its drain is cheap —
            # the gpsimd queue's final drain is not.
            nc.sync.dma_start(out=out[rows, :], in_=ot[:])
        else:
            nc.gpsimd.dma_start(out=out[rows, :], in_=ot[:])


def _mha_kernel(tc, out, xT, wqT, wkT, wvT, woT):
    nc = tc.nc
    with ExitStack() as ctx:
        singles = ctx.enter_context(tc.tile_pool(name="singles", bufs=1))

        # Weights ride the scalar-engine DMA queue (4 triggers, issued
        # before any exp work) so they flow in parallel with the first x
        # chunk's DMAs on the sync queue.
        w_sb = {}
        for name, ap in (("wq", wqT), ("wk", wkT), ("wv", wvT)):
            t = singles.tile([128, NE, 128], F16, tag=f"w_{name}",
                             name=f"w_{name}")
            nc.scalar.dma_start(out=t[:],
                                in_=ap.rearrange("(e p) o -> p e o", p=128))
            w_sb[name] = t
        wo_sb = singles.tile([128, D], F16, tag="wo")
        nc.scalar.dma_start(out=wo_sb[:], in_=woT[:])

        ident = singles.tile([128, 64], F16, tag="ident")
        make_identity(nc, ident[0:64, 0:64])
        make_identity(nc, ident[64:128, 0:64])

        qT = singles.tile([128, BS], F16, tag="qT")
        kT = singles.tile([128, BS], F16, tag="kT")
        v_aug = singles.tile([128, B * H_LOCAL, NTB, 65], F16, tag="v_aug")
        ones = singles.tile([128, 1], F16, tag="ones")
        nc.vector.memset(ones[:], 1.0)
        nc.vector.tensor_copy(
            v_aug[:, :, :, 64:65],
            ones[:].to_broadcast((128, B * H_LOCAL, NTB, 1)))
        y_cT = singles.tile([128, BS], F16, tag="y_cT")

        pools = {
            "x": ctx.enter_context(tc.tile_pool(name="x_pool", bufs=3)),
            "vt": ctx.enter_context(tc.tile_pool(name="vt_pool", bufs=2)),
            "pt": ctx.enter_context(tc.tile_pool(name="pt_pool", bufs=3)),
            "osb": ctx.enter_context(tc.tile_pool(name="osb_pool", bufs=4)),
            "nrm": ctx.enter_context(tc.tile_pool(name="nrm_pool", bufs=2)),
            "out": ctx.enter_context(tc.tile_pool(name="out_pool", bufs=3)),
            "mm_ps": ctx.enter_context(
                tc.tile_pool(name="mm_ps", bufs=2, space="PSUM")),
            "sc_ps": ctx.enter_context(
                tc.tile_pool(name="sc_ps", bufs=2, space="PSUM")),
            "o_ps": ctx.enter_context(
                tc.tile_pool(name="o_ps", bufs=1, space="PSUM")),
        }

        state = {}

        def proj(b, c):
            _proj_chunk(nc, pools, w_sb, ident, qT, kT, v_aug, xT, state,
                        b, c)

        def kq(b, c):
            _proj_kq(nc, pools, w_sb, qT, kT, xT, state, b, c)

        def vproj(b, c):
            _proj_v(nc, pools, w_sb, ident, v_aug, state, b, c)

        def part(b, c, lo, hi):
            _attn_part(nc, pools, qT, kT, v_aug, state, b, c, lo, hi)

        def pv0(b, c):
            _attn_pv0(nc, pools, v_aug, state, b, c)

        def fin(b, c, last=False, two_wide=False, do_oproj=True):
            _attn_finish(nc, pools, wo_sb, v_aug, y_cT, out, state, b, c,
                         last=last, two_wide=two_wide, do_oproj=do_oproj)

        def oproj(b, c):
            _attn_oproj(nc, pools, wo_sb, y_cT, out, b, c, two_wide=True)

        # Warm the PE p-state ramp while the first DMAs are in flight:
        # identity transposes depend only on `ident` (no DMA), and a
        # dummy reduce releases the PSUM slot before projections need it.
        warm = pools["mm_ps"].tile([128, 16, 64], F16, tag="mm",
                                   name="warm")
        for w in range(16):
            nc.tensor.transpose(warm[0:64, w, :], ident[0:64, 0:64],
                                ident[0:64, 0:64])
        wsink = singles.tile([64, 16, 64], F16, tag="wsink")
        nc.vector.tensor_copy(wsink[:], warm[0:64, :, :])

        # Batch 0 pipeline fill: each projection chunk unlocks 4 key
        # blocks of chunk 0's attention plus borrowed exp-only blocks of
        # chunk 1, sized to keep the scalar engine saturated while the
        # next projection runs. Head-0 PV trails as soon as each chunk's
        # exps are all emitted; finish bursts trail one chunk; batch 1's
        # projections are spread through batch 0's attention.
        proj(0, 0)
        part(0, 0, 0, 4)
        proj(0, 1)
        part(0, 0, 4, 8)
        part(0, 1, 0, 5)
        proj(0, 2)
        part(0, 0, 8, 12)
        part(0, 1, 5, 9)
        proj(0, 3)
        part(0, 0, 12, 16)
        pv0(0, 0)
        part(0, 1, 9, 16)
        pv0(0, 1)
        part(0, 2, 0, 16)
        pv0(0, 2)
        fin(0, 0)
        kq(1, 0)
        part(0, 3, 0, 16)
        pv0(0, 3)
        vproj(1, 0)
        fin(0, 1)
        kq(1, 1)
        vproj(1, 1)
        fin(0, 2)
        kq(1, 2)
        kq(1, 3)
        vproj(1, 2)
        vproj(1, 3)
        fin(0, 3)
        part(1, 0, 0, 16)
        pv0(1, 0)
        part(1, 1, 0, 16)
        pv0(1, 1)
        fin(1, 0, two_wide=True)
        part(1, 2, 0, 16)
        pv0(1, 2)
        fin(1, 1, two_wide=True)
        part(1, 3, 0, 16)
        pv0(1, 3)
        fin(1, 2, two_wide=True)
        fin(1, 3, last=True)


def build_nc(n_cores=N_CORES):
    nc = bacc.Bacc("TRN2", target_bir_lowering=False, debug=False,
                   num_devices=n_cores)
    xT = nc.dram_tensor("xT", [D, BS], F16, kind="ExternalInput").ap()
    wqT = nc.dram_tensor("wqT", [D, 128], F16, kind="ExternalInput").ap()
    wkT = nc.dram_tensor("wkT", [D, 128], F16, kind="ExternalInput").ap()
    wvT = nc.dram_tensor("wvT", [D, 128], F16, kind="ExternalInput").ap()
    woT = nc.dram_tensor("woT", [128, D], F16, kind="ExternalInput").ap()
    out = nc.dram_tensor("out", [BS, D], F16, kind="ExternalOutput").ap()
    with tile.TileContext(nc) as tc:
        _mha_kernel(tc, out, xT, wqT, wkT, wvT, woT)
    nc.compile()
    return nc


def make_in_maps(inputs, Wq, Wk, Wv, Wo, n_cores=N_CORES):
    x = np.asarray(inputs, dtype=np.float32).reshape(BS, D)
    xT = np.ascontiguousarray(x.T).astype(np.float16)
    Wq, Wk, Wv, Wo = (np.asarray(w, dtype=np.float32)
                      for w in (Wq, Wk, Wv, Wo))
    maps = []
    for c in range(n_cores):
        sl = slice(c * 128, (c + 1) * 128)
        maps.append({
            "xT": xT,
            "wqT": np.ascontiguousarray(Wq[sl, :].T).astype(np.float16),
            "wkT": np.ascontiguousarray(Wk[sl, :].T).astype(np.float16),
            "wvT": np.ascontiguousarray(Wv[sl, :].T).astype(np.float16),
            "woT": np.ascontiguousarray(Wo[:, sl].T).astype(np.float16),
        })
    return maps


_NC_CACHE = None


def run(inputs, Wq, Wk, Wv, Wo, trace=False):
    """Shard, run on the 8 NeuronCores, and unshard. Returns
    (output [B,S,D] float32, BassKernelResults)."""
    global _NC_CACHE
    from concourse.bass_utils import run_bass_kernel_spmd
    if _NC_CACHE is None:
        _NC_CACHE = build_nc()
    maps = make_in_maps(inputs, Wq, Wk, Wv, Wo)
    res = run_bass_kernel_spmd(_NC_CACHE, maps, list(range(N_CORES)),
                               trace=trace)
    acc = np.zeros((BS, D), dtype=np.float32)
    for rmap in res.results:
        acc += rmap["out"].astype(np.float32)
    return acc.reshape(B, S, D), res


def kernel(inputs, Wq, Wk, Wv, Wo):
    out, _ = run(inputs, Wq, Wk, Wv, Wo, trace=False)
    return out
